# revision 13
# baseline (speedup 1.0000x reference)
"""CIN (Compressed Interaction Network) forward kernel for 8 Trainium2 NeuronCores.

Reference computation (per batch b, embedding dim d):
    x0 = inputs[b, :, d]                 # [F=39]
    h0 = x0
    for k in 0..2:
        z  = outer(x0, h_{k})            # [F * Hk]
        h_{k+1} = z @ Wk + bk            # [256]
    out[b] = concat_k sum_d h_{k+1}      # [768]

Strategy: data-parallel over batch (64 per core).  Per core, rows r = (b, d)
are 2048 GEMM rows.  Layers 0/1 run as Khatri-Rao (DVE) + GEMM (PE) with
everything transposed: x0T[f, r], hT[u, r]; weights are the stationary
operand, z tiles the moving operand (fp16, N=512 moving columns).  Layer 0
exploits the symmetry of outer(x0, x0): only the 780 pairs (i <= j) are
computed, with W0 rows folded host-side (W[i,j] + W[j,i]), packed into 7
k-tiles whose xi/xj operand tiles are gathered host-side (xij).

Layer 2's output is only ever consumed as sum_d h3, so the full
[2048, 9984] @ [9984, 256] GEMM is replaced by the pooled identity
    out3[b, u] = sum_{i,j} (x0[b] @ h2[b]^T)[i, j] * W2[(i,j), u]
which contracts over d=32 first:
  1. PE-transpose h2 [256, 2048] -> h2t[r, j] (32 [128,128] transposes)
  2. step A: G'[j, (i,b)] = h2t^T @ x0-blockdiag   (32 matmuls, N=156)
  3. step B: out3[b, u]   = sum_{i,jh} G'^T @ W2    (78 matmuls, N=256)
This is ~28x fewer FLOPs than the direct layer 2 and needs no z2 tiles
(halves DVE work) and no layer-2 xi broadcasts (-20MB DMA).
"""

import os
import sys

import numpy as np

for _p in ("/opt/trn_rl_repo", "/root/.axon_site/_ro/trn_rl_repo"):
    if os.path.isdir(_p) and _p not in sys.path:
        sys.path.insert(0, _p)

N_CORES = 8
B, F, D = 512, 39, 32
U = 256
BL = B // N_CORES          # 64 batches per core
R = BL * D                 # 2048 GEMM rows per core
NB = 512                   # matmul moving free-dim (one PSUM bank of fp32)
NRB = R // NB              # 4 row blocks
NBE = 512                  # evacuation / boundary-slice granularity
K0S = F * (F + 1) // 2     # 780 symmetric pairs (i <= j)
KT0 = 7                    # layer-0 k-tiles: 6 full + one 12-row remainder
K0_LAST = K0S - 6 * 128    # 12
FP = 42                    # x0r replication factor (one row per broadcast read)
K12 = F * U                # 9984
KT12 = K12 // 128          # 78 k-tiles; kt = (i, half)
NWARM = 30                 # PE warm-up matmuls (rotating PSUM banks)

DT = "float16"             # device compute dtype for z / W / h ("float16" | "bfloat16")

_prog_cache = {}


def _np_dt():
    import ml_dtypes

    return np.float16 if DT == "float16" else ml_dtypes.bfloat16


def _build_program():
    import concourse.mybir as mybir
    from concourse import bacc, tile

    dt = mybir.dt
    cdt = getattr(dt, DT)
    f32 = dt.float32

    nc = bacc.Bacc(
        "TRN2", target_bir_lowering=False, debug=False, num_devices=N_CORES
    )
    # x0 rows each replicated 42x in DRAM: broadcast DMAs read distinct
    # addresses (HBM bank spread) instead of hammering one 4KB row.
    x0r_p = nc.declare_dram_parameter("x0r", [F * FP, R], cdt, isOutput=False)
    # layer-0 operand tiles, host-gathered: [t, s] with s=0 -> x0[i(p)],
    # s=1 -> x0[j(p)] for pair slot p of symmetric k-tile t
    xij0_p = nc.declare_dram_parameter("xij0", [2 * KT0, 128, R], cdt, isOutput=False)
    w0_p = nc.declare_dram_parameter("w0", [128, KT0, U], cdt, isOutput=False)
    w1_p = nc.declare_dram_parameter("w1", [128, KT12, U], cdt, isOutput=False)
    # W2 repacked for step B: w2[j, jh, i*256 + u] = W2[(i*256 + jh*128 + j), u]
    w2_p = nc.declare_dram_parameter("w2", [128, 2, K12], cdt, isOutput=False)
    # x0 block-diagonal for step A: xd[p, bb*156 + i*4 + bi]
    #   = x0[b = bb*4 + bi, i, d = p%32] if p//32 == bi else 0
    xd_p = nc.declare_dram_parameter("xd", [128, 16 * 156], cdt, isOutput=False)
    ident_p = nc.declare_dram_parameter("ident", [128, 128], cdt, isOutput=False)
    bias_p = nc.declare_dram_parameter("bias", [128, 4], f32, isOutput=False)
    out_p = nc.declare_dram_parameter("out", [128, 4, BL], f32, isOutput=True)
    out3_p = nc.declare_dram_parameter("out3", [BL, U], f32, isOutput=True)

    with tile.TileContext(nc) as tc:
        with (
            tc.tile_pool(name="const", bufs=1) as constp,
            tc.tile_pool(name="wpool", bufs=1) as wpool,
            tc.tile_pool(name="xb", bufs=5) as xbp,
            tc.tile_pool(name="zp", bufs=4) as zp,
            tc.tile_pool(name="hp", bufs=1) as hp,
            tc.tile_pool(name="psum", bufs=1, space="PSUM") as psp,
        ):
            # broadcast DMAs source from DRAM (re-reading one SBUF partition
            # 128x serializes on its port) and alternate trigger engines so
            # both dynamic HW queues run in parallel.
            bcast_n = [0]

            def bcast(dst, src_ap):
                eng = nc.sync if bcast_n[0] % 2 == 0 else nc.scalar
                bcast_n[0] += 1
                eng.dma_start(dst, src_ap)

            out_sb = constp.tile([128, 4, BL], f32, tag="out")
            out3_sb = constp.tile([BL, U], f32, tag="out3")
            h_tiles = {
                (l, c): hp.tile([128, R], cdt, tag=f"h{l}{c}", name=f"h{l}{c}")
                for l in range(2)
                for c in range(2)
            }
            # warm-up matmuls read this region before layer 0 writes it
            nc.vector.memset(h_tiles[(0, 0)][:, :NBE], 0.0)
            h2t = constp.tile([128, 16, U], cdt, tag="h2t")
            g_sb = constp.tile([128, 2, 16 * 156], cdt, tag="g")
            g_v = g_sb.rearrange("p two (i w) -> p two i w", i=F)
            xd_sb = constp.tile([128, 16 * 156], cdt, tag="xd")
            ident = constp.tile([128, 128], cdt, tag="ident")

            # ---- prologue, hand-ordered so the critical path clears first.
            # The fat xij operand tiles stream on the sync+scalar HW queues;
            # weights/constants ride the otherwise-idle GpSimd queue so the
            # three dynamic DMA queues pull HBM in parallel.
            xi0_tiles = []

            def xij0_dma(t, split=False):
                xi = xbp.tile([128, R], cdt, tag="xi", name=f"x0i{t}", bufs=14)
                xj = xbp.tile([128, R], cdt, tag="xi", name=f"x0j{t}", bufs=14)
                if split:
                    # first-consumed tiles go in small pieces so their
                    # completion semaphores fire early
                    nc.sync.dma_start(xi[:64, :], xij0_p[2 * t, :64, :])
                    nc.scalar.dma_start(xj[:64, :], xij0_p[2 * t + 1, :64, :])
                    nc.sync.dma_start(xi[64:, :], xij0_p[2 * t, 64:, :])
                    nc.scalar.dma_start(xj[64:, :], xij0_p[2 * t + 1, 64:, :])
                else:
                    bcast(xi[:, :], xij0_p[2 * t, :, :])
                    bcast(xj[:, :], xij0_p[2 * t + 1, :, :])
                xi0_tiles.append((xi, xj))

            w0 = wpool.tile([128, KT0, U], cdt, tag="w0")
            w1 = wpool.tile([128, KT12, U], cdt, tag="w1")
            bias = constp.tile([128, 4], f32, tag="bias")

            w1_chunks = list(range(0, KT12, 13))
            xij0_dma(0, split=True)
            nc.gpsimd.dma_start(w0[:, :2, :], w0_p[:, :2, :])
            nc.gpsimd.dma_start(bias[:, :], bias_p[:, :])
            xij0_dma(1)
            nc.gpsimd.dma_start(w0[:, 2:, :], w0_p[:, 2:, :])
            xij0_dma(2)
            xij0_dma(3)
            nc.gpsimd.dma_start(w1[:, 0:13, :], w1_p[:, 0:13, :])
            xij0_dma(4)
            xij0_dma(5)
            nc.gpsimd.dma_start(w1[:, 13:26, :], w1_p[:, 13:26, :])
            xij0_dma(6)
            nc.gpsimd.dma_start(w1[:, 26:39, :], w1_p[:, 26:39, :])

            # ---- PE warm-up: the HAM clock gate needs ~3.4us of sustained
            # matmul activity to unthrottle 1.2 -> 2.4 GHz.  Startup is
            # DMA-bound anyway, so burn dummy matmuls on garbage SBUF data
            # into rotating PSUM banks (pipelined, ~260ns apiece); the first
            # real accumulation starts with start=True, which clears the bank.
            warm_ps = [
                psp.tile([128, NB], f32, tag=f"ps_0_{r}", name=f"warm_{r}")
                for r in range(NRB)
            ]
            for k in range(NWARM):
                nc.tensor.matmul(
                    warm_ps[k % 4][:, :],
                    h_tiles[(0, 0)][:, :128],
                    h_tiles[(0, 0)][:, :NB],
                    start=True,
                    stop=True,
                )

            def make_x(i, nm):
                t = xbp.tile([128, R], cdt, tag="xi", name=nm, bufs=14)
                bcast(
                    t[:, :],
                    x0r_p[i * FP : i * FP + 32, :]
                    .unsqueeze(1)
                    .to_broadcast((32, 4, R)),
                )
                return t

            l1_pre = {i: make_x(i, f"l1x{i}") for i in (0, 1)}

            def do_layer(l, w_t, z_fn, kt_n, kt_hook=None):
                ps = [
                    [
                        psp.tile([128, NB], f32, tag=f"ps_{c}_{r}", name=f"ps_{c}_{r}")
                        for r in range(NRB)
                    ]
                    for c in range(2)
                ]
                for kt in range(kt_n):
                    if kt_hook is not None:
                        kt_hook(kt)
                    klen, z_t = z_fn(kt)
                    for c in range(2):
                        lhsT = w_t[:klen, kt, c * 128 : (c + 1) * 128]
                        for r in range(NRB):
                            nc.tensor.matmul(
                                ps[c][r][:, :],
                                lhsT,
                                z_t[:klen, r * NB : (r + 1) * NB],
                                start=(kt == 0),
                                stop=(kt == kt_n - 1),
                            )
                # evacuations gate the next phase's tensor ops and free the
                # PSUM banks.  c=0 on DVE (same-engine gate for the next
                # layer's first TTs), c=1 on the otherwise-idle Scalar engine
                # so both halves evacuate in parallel at the boundary.
                for c in range(2):
                    for rq in range(NRB):
                        src = ps[c][rq][:, :]
                        dst = h_tiles[(l, c)][:, rq * NB : (rq + 1) * NB]
                        if c == 0:
                            nc.vector.tensor_scalar_add(
                                dst, src, bias[:, l * 2 + c : l * 2 + c + 1]
                            )
                        else:
                            nc.scalar.activation(
                                dst,
                                src,
                                mybir.ActivationFunctionType.Identity,
                                bias=bias[:, l * 2 + c : l * 2 + c + 1],
                            )

            def h_reduce(l):
                # d-sum for the pooled output of layers 0/1 (layer 0's is
                # deferred into layer 1, layer 1's into the step-B window,
                # when the DVE is otherwise idle).
                for c in range(2):
                    nc.vector.tensor_reduce(
                        out_sb[:, l * 2 + c, :],
                        h_tiles[(l, c)].rearrange("p (b d) -> p b d", d=D),
                        axis=mybir.AxisListType.X,
                        op=mybir.AluOpType.add,
                    )

            # ---- layer 0 (symmetric): k-tile t holds pair slots p with
            # (i, j) = pairs[t*128 + p]; W0 rows are host-folded so only
            # i <= j pairs are computed.  The last tile has 12 live rows. ----
            def z_layer0(kt):
                klen = 128 if kt < KT0 - 1 else K0_LAST
                xi, xj = xi0_tiles[kt]
                z_t = zp.tile([128, R], cdt, tag="z")
                nc.vector.tensor_mul(z_t[:klen, :], xi[:klen, :], xj[:klen, :])
                return klen, z_t

            do_layer(0, w0, z_layer0, KT0)

            # ---- layer 1: z[(i, j), r] = x0[i, r] * h1[j, r], k = i*256 + j ----
            def z_layer12(l, premade):
                xcur = [None]

                def fn(kt):
                    i, half = kt // 2, kt % 2
                    if half == 0:
                        if i in premade:
                            xcur[0] = premade[i]
                        else:
                            xcur[0] = make_x(i, "xi")
                    z_t = zp.tile([128, R], cdt, tag="z")
                    if kt < 2:
                        # boundary pipelining: slice-wise TT so each matmul's z
                        # slice is ready right after its h evacuation lands
                        for rq in range(4):
                            nc.vector.tensor_mul(
                                z_t[:, rq * NBE : (rq + 1) * NBE],
                                xcur[0][:, rq * NBE : (rq + 1) * NBE],
                                h_tiles[(l - 1, half)][:, rq * NBE : (rq + 1) * NBE],
                            )
                    else:
                        nc.vector.tensor_mul(
                            z_t[:, :], xcur[0][:, :], h_tiles[(l - 1, half)][:, :]
                        )
                    return 128, z_t

                return fn

            w2sb = wpool.tile([128, 2, K12], cdt, tag="w2")

            # stream the rest of W1 + all of W2 + tail constants at spread
            # points in layer 1; w1 chunk c is consumed starting at kt = 13c.
            w2_sched = {26: 0, 34: 1, 42: 2, 50: 3, 58: 4, 64: 5}
            w1_sched = {2: 3, 12: 4, 22: 5}

            def w_hook(kt):
                if kt in w1_sched:
                    c = w1_sched[kt]
                    lo = w1_chunks[c]
                    (nc.sync if c % 2 else nc.scalar).dma_start(
                        w1[:, lo : lo + 13, :], w1_p[:, lo : lo + 13, :]
                    )
                if kt in w2_sched:
                    c = w2_sched[kt]
                    lo = c * 1664
                    (nc.sync if c % 2 else nc.scalar).dma_start(
                        w2sb[:, :, lo : lo + 1664], w2_p[:, :, lo : lo + 1664]
                    )
                if kt == 30:
                    nc.sync.dma_start(xd_sb[:, :], xd_p[:, :])
                if kt == 36:
                    nc.scalar.dma_start(ident[:, :], ident_p[:, :])
                if kt == 4:
                    h_reduce(0)   # deferred layer-0 d-sum, on GpSimd
                if kt == 6:
                    nc.sync.dma_start(out_p[:, 0:2, :], out_sb[:, 0:2, :])

            do_layer(1, w1, z_layer12(1, l1_pre), KT12, kt_hook=w_hook)

            # ---- layer 2 tail: pooled-output trick ----
            # h2 [256(u), 2048(r)] -> h2t[r, bb, u] via 32 PE transposes.
            # PSUM tags are reused from the (now free) layer-1 banks.
            def transpose_pair(bb):
                psT = psp.tile(
                    [128, 2 * 128], cdt, tag=f"ps_0_{bb % 4}", name=f"psT{bb}"
                )
                for c in range(2):
                    nc.tensor.matmul(
                        psT[:, c * 128 : (c + 1) * 128],
                        h_tiles[(1, c)][:, bb * 128 : (bb + 1) * 128],
                        ident[:, :],
                        is_transpose=True,
                    )
                # evacuate both c halves as one [128, 256] copy
                if bb % 2 == 0:
                    nc.vector.tensor_scalar_add(h2t[:, bb, :], psT[:, :], 0.0)
                else:
                    nc.scalar.copy(h2t[:, bb, :], psT[:, :])

            def step_a(bb):
                psA = psp.tile(
                    [128, 2 * 156], f32, tag=f"ps_1_{bb % 4}", name=f"psA{bb}"
                )
                for jh in range(2):
                    nc.tensor.matmul(
                        psA[:, jh * 156 : (jh + 1) * 156],
                        h2t[:, bb, jh * 128 : (jh + 1) * 128],
                        xd_sb[:, bb * 156 : (bb + 1) * 156],
                        start=True,
                        stop=True,
                    )
                # scatter into i-major G layout (col = i*64 + bb*4 + bi) so
                # step B's stationary slices are contiguous (BIR requires a
                # single free dim on the weights AP)
                for jh in range(2):
                    src = psA[:, jh * 156 : (jh + 1) * 156].rearrange(
                        "p (i w) -> p i w", i=F
                    )
                    dst = g_v[:, jh, :, bb * 4 : (bb + 1) * 4]
                    if bb % 2 == 0:
                        nc.scalar.copy(dst, src)
                    else:
                        nc.vector.tensor_scalar_add(dst, src, 0.0)

            transpose_pair(0)
            transpose_pair(1)
            for bb in range(2, 16):
                transpose_pair(bb)
                step_a(bb - 2)
            step_a(14)
            step_a(15)
            h_reduce(1)   # layer-1 d-sum on GpSimd, in parallel with the tail
            nc.sync.dma_start(out_p[:, 2:4, :], out_sb[:, 2:4, :])

            # step B: out3[b, u] = sum_{i, jh} G'[jh][:, (i, b)]^T
            #                                   @ W2[jh][:, i*256:(i+1)*256]
            psB = psp.tile([BL, U], f32, tag="ps_0_0", name="psB")
            for i in range(F):
                for jh in range(2):
                    nc.tensor.matmul(
                        psB[:, :],
                        g_sb[:, jh, i * BL : (i + 1) * BL],
                        w2sb[:, jh, i * U : (i + 1) * U],
                        start=(i == 0 and jh == 0),
                        stop=(i == F - 1 and jh == 1),
                    )
            nc.vector.tensor_scalar_add(out3_sb[:, :], psB[:, :], 0.0)
            nc.sync.dma_start(out3_p[:, :], out3_sb[:, :])

    nc.compile()
    return nc


def _get_program():
    if "nc" not in _prog_cache:
        _prog_cache["nc"] = _build_program()
    return _prog_cache["nc"]


def _prep_maps(inputs):
    cdt = _np_dt()
    x = np.asarray(inputs["inputs"], np.float32)          # [512, 39, 32]
    Ws = [np.asarray(inputs[f"W{k}"], np.float32) for k in range(3)]
    bs = [np.asarray(inputs[f"b{k}"], np.float32) for k in range(3)]

    # layer-0 symmetric packing: pair slot t*128 + p -> (i, j), i <= j,
    # with the j > i weight row folded in host-side
    pairs = [(i, j) for i in range(F) for j in range(i, F)]
    w0r = Ws[0].reshape(F, F, U)
    w0t = np.zeros((KT0, 128, U), np.float32)
    for s, (i, j) in enumerate(pairs):
        t, p = divmod(s, 128)
        w0t[t, p] = w0r[i, j] if i == j else w0r[i, j] + w0r[j, i]
    w0_tiled = np.ascontiguousarray(w0t.transpose(1, 0, 2).astype(cdt))
    w1_tiled = np.ascontiguousarray(
        Ws[1].reshape(KT12, 128, U).transpose(1, 0, 2).astype(cdt)
    )
    # step-B W2 layout: w2[j, jh, i*256 + u] = W2[(i, jh*128 + j), u]
    w2r = Ws[2].reshape(F, 2, 128, U)                     # [i, jh, j, u]
    w2_tiled = np.ascontiguousarray(
        w2r.transpose(2, 1, 0, 3).reshape(128, 2, F * U).astype(cdt)
    )
    ident = np.ascontiguousarray(np.eye(128, dtype=np.float32).astype(cdt))
    bias = np.zeros((128, 4), np.float32)
    for l in range(2):
        for c in range(2):
            bias[:, l * 2 + c] = bs[l][c * 128 : (c + 1) * 128]

    pr_i = np.array([p[0] for p in pairs])
    pr_j = np.array([p[1] for p in pairs])
    in_maps = []
    for core in range(N_CORES):
        xs = x[core * BL : (core + 1) * BL]               # [64, 39, 32]
        x0T = xs.transpose(1, 0, 2).reshape(F, R).astype(cdt)
        x0r = np.ascontiguousarray(np.repeat(x0T, FP, axis=0))
        xi_all = np.zeros((KT0 * 128, R), cdt)
        xj_all = np.zeros((KT0 * 128, R), cdt)
        xi_all[: len(pairs)] = x0T[pr_i]
        xj_all[: len(pairs)] = x0T[pr_j]
        xij = np.zeros((2 * KT0, 128, R), cdt)
        xij[0::2] = xi_all.reshape(KT0, 128, R)
        xij[1::2] = xj_all.reshape(KT0, 128, R)
        xij = np.ascontiguousarray(xij)
        # step-A x0 block-diagonal: xd[p, bb*156 + i*4 + bi]
        #   = xs[bb*4 + bi, i, p % 32] when p // 32 == bi
        xd = np.zeros((128, 16, F, 4), np.float32)
        xsr = xs.reshape(16, 4, F, D)                     # [bb, bi, i, d]
        for bi in range(4):
            xd[bi * 32 : (bi + 1) * 32, :, :, bi] = xsr[:, bi].transpose(2, 0, 1)
        xd = np.ascontiguousarray(xd.reshape(128, 16 * 156).astype(cdt))
        in_maps.append(
            {
                "xij0": xij,
                "x0r": x0r,
                "w0": w0_tiled,
                "w1": w1_tiled,
                "w2": w2_tiled,
                "xd": xd,
                "ident": ident,
                "bias": bias,
            }
        )
    return in_maps, bs


def _finish_output(results, bs):
    outs = []
    for core in range(N_CORES):
        o = np.asarray(results[core]["out"], np.float32)  # [128, 4, 64]
        o3 = np.asarray(results[core]["out3"], np.float32)  # [64, 256]
        full = np.concatenate(
            [o.transpose(2, 1, 0).reshape(BL, 512), o3], axis=1
        )
        outs.append(full)
    out = np.concatenate(outs, axis=0)
    for l in range(3):
        out[:, l * U : (l + 1) * U] += D * bs[l]
    return np.ascontiguousarray(out.astype(np.float32))


def kernel(**inputs) -> np.ndarray:
    from concourse.bass_utils import run_bass_kernel_spmd

    in_maps, bs = _prep_maps(inputs)
    nc = _get_program()
    res = run_bass_kernel_spmd(nc, in_maps, list(range(N_CORES))).results
    return _finish_output(res, bs)


# revision 14
# speedup vs baseline: 1.0351x; 1.0351x over previous
"""CIN (Compressed Interaction Network) forward kernel for 8 Trainium2 NeuronCores.

Reference computation (per batch b, embedding dim d):
    x0 = inputs[b, :, d]                 # [F=39]
    h0 = x0
    for k in 0..2:
        z  = outer(x0, h_{k})            # [F * Hk]
        h_{k+1} = z @ Wk + bk            # [256]
    out[b] = concat_k sum_d h_{k+1}      # [768]

Strategy: data-parallel over batch (64 per core).  Per core, rows r = (b, d)
are 2048 GEMM rows.  Layers 0/1 run as Khatri-Rao (DVE) + GEMM (PE) with
everything transposed: x0T[f, r], hT[u, r]; weights are the stationary
operand, z tiles the moving operand (fp16, N=512 moving columns).  Layer 0
exploits the symmetry of outer(x0, x0): only the 780 pairs (i <= j) are
computed, with W0 rows folded host-side (W[i,j] + W[j,i]), packed into 7
k-tiles whose xi/xj operand tiles are gathered host-side (xij).

Layer 2's output is only ever consumed as sum_d h3, so the full
[2048, 9984] @ [9984, 256] GEMM is replaced by the pooled identity
    out3[b, u] = sum_{i,j} (x0[b] @ h2[b]^T)[i, j] * W2[(i,j), u]
which contracts over d=32 first:
  1. PE-transpose h2 [256, 2048] -> h2t[r, j] (32 [128,128] transposes)
  2. step A: G'[j, (i,b)] = h2t^T @ x0-blockdiag   (32 matmuls, N=156)
  3. step B: out3[b, u]   = sum_{i,jh} G'^T @ W2    (78 matmuls, N=256)
This is ~28x fewer FLOPs than the direct layer 2 and needs no z2 tiles
(halves DVE work) and no layer-2 xi broadcasts (-20MB DMA).
"""

import os
import sys

import numpy as np

for _p in ("/opt/trn_rl_repo", "/root/.axon_site/_ro/trn_rl_repo"):
    if os.path.isdir(_p) and _p not in sys.path:
        sys.path.insert(0, _p)

N_CORES = 8
B, F, D = 512, 39, 32
U = 256
BL = B // N_CORES          # 64 batches per core
R = BL * D                 # 2048 GEMM rows per core
NB = 512                   # matmul moving free-dim (one PSUM bank of fp32)
NRB = R // NB              # 4 row blocks
NBE = 512                  # evacuation / boundary-slice granularity
K0S = F * (F + 1) // 2     # 780 symmetric pairs (i <= j)
KT0 = 7                    # layer-0 k-tiles: 6 full + one 12-row remainder
K0_LAST = K0S - 6 * 128    # 12
FP = 42                    # x0r replication factor (one row per broadcast read)
K12 = F * U                # 9984
KT12 = K12 // 128          # 78 k-tiles; kt = (i, half)
NWARM = 16                 # PE warm-up matmuls (rotating PSUM banks)

DT = "float16"             # device compute dtype for z / W / h ("float16" | "bfloat16")

_prog_cache = {}


def _np_dt():
    import ml_dtypes

    return np.float16 if DT == "float16" else ml_dtypes.bfloat16


def _build_program():
    import concourse.mybir as mybir
    from concourse import bacc, tile

    dt = mybir.dt
    cdt = getattr(dt, DT)
    f32 = dt.float32

    nc = bacc.Bacc(
        "TRN2", target_bir_lowering=False, debug=False, num_devices=N_CORES
    )
    # x0 rows each replicated 42x in DRAM: broadcast DMAs read distinct
    # addresses (HBM bank spread) instead of hammering one 4KB row.
    x0r_p = nc.declare_dram_parameter("x0r", [F * FP, R], cdt, isOutput=False)
    # layer-0 operand tiles, host-gathered: [t, s] with s=0 -> x0[i(p)],
    # s=1 -> x0[j(p)] for pair slot p of symmetric k-tile t
    xij0_p = nc.declare_dram_parameter("xij0", [2 * KT0, 128, R], cdt, isOutput=False)
    w0_p = nc.declare_dram_parameter("w0", [128, KT0, U], cdt, isOutput=False)
    w1_p = nc.declare_dram_parameter("w1", [128, KT12, U], cdt, isOutput=False)
    # W2 repacked for step B: w2[j, jh, i*256 + u] = W2[(i*256 + jh*128 + j), u]
    w2_p = nc.declare_dram_parameter("w2", [128, 2, K12], cdt, isOutput=False)
    # x0 block-diagonal for step A: xd[p, bb*156 + i*4 + bi]
    #   = x0[b = bb*4 + bi, i, d = p%32] if p//32 == bi else 0
    xd_p = nc.declare_dram_parameter("xd", [128, 16 * 156], cdt, isOutput=False)
    ident_p = nc.declare_dram_parameter("ident", [128, 128], cdt, isOutput=False)
    bias_p = nc.declare_dram_parameter("bias", [128, 4], f32, isOutput=False)
    out_p = nc.declare_dram_parameter("out", [128, 4, BL], f32, isOutput=True)
    out3_p = nc.declare_dram_parameter("out3", [BL, U], f32, isOutput=True)

    with tile.TileContext(nc) as tc:
        with (
            tc.tile_pool(name="const", bufs=1) as constp,
            tc.tile_pool(name="wpool", bufs=1) as wpool,
            tc.tile_pool(name="xb", bufs=5) as xbp,
            tc.tile_pool(name="zp", bufs=4) as zp,
            tc.tile_pool(name="hp", bufs=1) as hp,
            tc.tile_pool(name="psum", bufs=1, space="PSUM") as psp,
        ):
            # broadcast DMAs source from DRAM (re-reading one SBUF partition
            # 128x serializes on its port) and alternate trigger engines so
            # both dynamic HW queues run in parallel.
            bcast_n = [0]

            def bcast(dst, src_ap):
                eng = nc.sync if bcast_n[0] % 2 == 0 else nc.scalar
                bcast_n[0] += 1
                eng.dma_start(dst, src_ap)

            out_sb = constp.tile([128, 4, BL], f32, tag="out")
            out3_sb = constp.tile([BL, U], f32, tag="out3")
            h_tiles = {
                (l, c): hp.tile([128, R], cdt, tag=f"h{l}{c}", name=f"h{l}{c}")
                for l in range(2)
                for c in range(2)
            }
            # warm-up matmuls read this region before layer 0 writes it
            nc.vector.memset(h_tiles[(0, 0)][:, :NBE], 0.0)
            h2t = constp.tile([128, 16, U], cdt, tag="h2t")
            g_sb = constp.tile([128, 2, 16 * 156], cdt, tag="g")
            g_v = g_sb.rearrange("p two (i w) -> p two i w", i=F)
            xd_sb = constp.tile([128, 16 * 156], cdt, tag="xd")
            ident = constp.tile([128, 128], cdt, tag="ident")

            # ---- prologue, hand-ordered so the critical path clears first.
            # The fat xij operand tiles stream on the sync+scalar HW queues;
            # weights/constants ride the otherwise-idle GpSimd queue so the
            # three dynamic DMA queues pull HBM in parallel.
            xi0_tiles = []

            def xij0_dma(t, split=False):
                xi = xbp.tile([128, R], cdt, tag="xi", name=f"x0i{t}", bufs=14)
                xj = xbp.tile([128, R], cdt, tag="xi", name=f"x0j{t}", bufs=14)
                if split:
                    # first-consumed tiles go in small pieces so their
                    # completion semaphores fire early
                    nc.sync.dma_start(xi[:64, :], xij0_p[2 * t, :64, :])
                    nc.scalar.dma_start(xj[:64, :], xij0_p[2 * t + 1, :64, :])
                    nc.sync.dma_start(xi[64:, :], xij0_p[2 * t, 64:, :])
                    nc.scalar.dma_start(xj[64:, :], xij0_p[2 * t + 1, 64:, :])
                else:
                    bcast(xi[:, :], xij0_p[2 * t, :, :])
                    bcast(xj[:, :], xij0_p[2 * t + 1, :, :])
                xi0_tiles.append((xi, xj))

            w0 = wpool.tile([128, KT0, U], cdt, tag="w0")
            w1 = wpool.tile([128, KT12, U], cdt, tag="w1")
            bias = constp.tile([128, 4], f32, tag="bias")

            w1_chunks = list(range(0, KT12, 13))
            xij0_dma(0, split=True)
            nc.gpsimd.dma_start(w0[:, :2, :], w0_p[:, :2, :])
            nc.gpsimd.dma_start(bias[:, :], bias_p[:, :])
            xij0_dma(1, split=True)
            nc.gpsimd.dma_start(w0[:, 2:, :], w0_p[:, 2:, :])
            xij0_dma(2, split=True)
            nc.gpsimd.dma_start(w1[:, 0:13, :], w1_p[:, 0:13, :])
            xij0_dma(3, split=True)
            xij0_dma(4, split=True)
            xij0_dma(5, split=True)
            xij0_dma(6, split=True)

            # ---- PE warm-up: the HAM clock gate needs ~3.4us of sustained
            # matmul activity to unthrottle 1.2 -> 2.4 GHz.  Startup is
            # DMA-bound anyway, so burn dummy matmuls on garbage SBUF data
            # into rotating PSUM banks (pipelined, ~260ns apiece); the first
            # real accumulation starts with start=True, which clears the bank.
            warm_ps = [
                psp.tile([128, NB], f32, tag=f"ps_0_{r}", name=f"warm_{r}")
                for r in range(NRB)
            ]
            for k in range(NWARM):
                nc.tensor.matmul(
                    warm_ps[k % 4][:, :],
                    h_tiles[(0, 0)][:, :128],
                    h_tiles[(0, 0)][:, :NB],
                    start=True,
                    stop=True,
                )

            def make_x(i, nm, eng=None):
                t = xbp.tile([128, R], cdt, tag="xi", name=nm, bufs=14)
                src_ap = (
                    x0r_p[i * FP : i * FP + 32, :]
                    .unsqueeze(1)
                    .to_broadcast((32, 4, R))
                )
                if eng is None:
                    bcast(t[:, :], src_ap)
                else:
                    eng.dma_start(t[:, :], src_ap)
                return t

            l1_pre = {i: make_x(i, f"l1x{i}", eng=nc.gpsimd) for i in (0, 1)}

            def do_layer(l, w_t, z_fn, kt_n, kt_hook=None):
                ps = [
                    [
                        psp.tile([128, NB], f32, tag=f"ps_{c}_{r}", name=f"ps_{c}_{r}")
                        for r in range(NRB)
                    ]
                    for c in range(2)
                ]
                for kt in range(kt_n):
                    if kt_hook is not None:
                        kt_hook(kt)
                    klen, z_t = z_fn(kt)
                    for c in range(2):
                        lhsT = w_t[:klen, kt, c * 128 : (c + 1) * 128]
                        for r in range(NRB):
                            nc.tensor.matmul(
                                ps[c][r][:, :],
                                lhsT,
                                z_t[:klen, r * NB : (r + 1) * NB],
                                start=(kt == 0),
                                stop=(kt == kt_n - 1),
                            )
                # evacuations gate the next phase's tensor ops and free the
                # PSUM banks.  c=0 on DVE (same-engine gate for the next
                # layer's first TTs), c=1 on the otherwise-idle Scalar engine
                # so both halves evacuate in parallel at the boundary.
                for c in range(2):
                    for rq in range(NRB):
                        src = ps[c][rq][:, :]
                        dst = h_tiles[(l, c)][:, rq * NB : (rq + 1) * NB]
                        if c == 0:
                            nc.vector.tensor_scalar_add(
                                dst, src, bias[:, l * 2 + c : l * 2 + c + 1]
                            )
                        else:
                            nc.scalar.activation(
                                dst,
                                src,
                                mybir.ActivationFunctionType.Identity,
                                bias=bias[:, l * 2 + c : l * 2 + c + 1],
                            )

            def h_reduce(l):
                # d-sum for the pooled output of layers 0/1 (layer 0's is
                # deferred into layer 1, layer 1's into the step-B window,
                # when the DVE is otherwise idle).
                for c in range(2):
                    nc.vector.tensor_reduce(
                        out_sb[:, l * 2 + c, :],
                        h_tiles[(l, c)].rearrange("p (b d) -> p b d", d=D),
                        axis=mybir.AxisListType.X,
                        op=mybir.AluOpType.add,
                    )

            # ---- layer 0 (symmetric): k-tile t holds pair slots p with
            # (i, j) = pairs[t*128 + p]; W0 rows are host-folded so only
            # i <= j pairs are computed.  The last tile has 12 live rows. ----
            def z_layer0(kt):
                klen = 128 if kt < KT0 - 1 else K0_LAST
                xi, xj = xi0_tiles[kt]
                z_t = zp.tile([128, R], cdt, tag="z")
                nc.vector.tensor_mul(z_t[:klen, :], xi[:klen, :], xj[:klen, :])
                return klen, z_t

            do_layer(0, w0, z_layer0, KT0)

            # ---- layer 1: z[(i, j), r] = x0[i, r] * h1[j, r], k = i*256 + j ----
            def z_layer12(l, premade):
                xcur = [None]

                def fn(kt):
                    i, half = kt // 2, kt % 2
                    if half == 0:
                        if i in premade:
                            xcur[0] = premade[i]
                        else:
                            xcur[0] = make_x(i, "xi")
                    z_t = zp.tile([128, R], cdt, tag="z")
                    if kt < 2:
                        # boundary pipelining: slice-wise TT so each matmul's z
                        # slice is ready right after its h evacuation lands
                        for rq in range(4):
                            nc.vector.tensor_mul(
                                z_t[:, rq * NBE : (rq + 1) * NBE],
                                xcur[0][:, rq * NBE : (rq + 1) * NBE],
                                h_tiles[(l - 1, half)][:, rq * NBE : (rq + 1) * NBE],
                            )
                    else:
                        nc.vector.tensor_mul(
                            z_t[:, :], xcur[0][:, :], h_tiles[(l - 1, half)][:, :]
                        )
                    return 128, z_t

                return fn

            w2sb = wpool.tile([128, 2, K12], cdt, tag="w2")

            # stream the rest of W1 + all of W2 + tail constants at spread
            # points in layer 1; w1 chunk c is consumed starting at kt = 13c.
            w2_sched = {26: 0, 34: 1, 42: 2, 50: 3, 58: 4, 64: 5}
            w1_sched = {0: 1, 4: 2, 10: 3, 18: 4, 28: 5}

            def w_hook(kt):
                if kt in w1_sched:
                    c = w1_sched[kt]
                    lo = w1_chunks[c]
                    (nc.sync if c % 2 else nc.scalar).dma_start(
                        w1[:, lo : lo + 13, :], w1_p[:, lo : lo + 13, :]
                    )
                if kt in w2_sched:
                    c = w2_sched[kt]
                    lo = c * 1664
                    (nc.sync if c % 2 else nc.scalar).dma_start(
                        w2sb[:, :, lo : lo + 1664], w2_p[:, :, lo : lo + 1664]
                    )
                if kt == 30:
                    nc.sync.dma_start(xd_sb[:, :], xd_p[:, :])
                if kt == 36:
                    nc.scalar.dma_start(ident[:, :], ident_p[:, :])
                if kt == 4:
                    h_reduce(0)   # deferred layer-0 d-sum, on GpSimd
                if kt == 6:
                    nc.sync.dma_start(out_p[:, 0:2, :], out_sb[:, 0:2, :])

            do_layer(1, w1, z_layer12(1, l1_pre), KT12, kt_hook=w_hook)

            # ---- layer 2 tail: pooled-output trick ----
            # h2 [256(u), 2048(r)] -> h2t[r, bb, u] via 32 PE transposes.
            # PSUM tags are reused from the (now free) layer-1 banks.
            def transpose_pair(bb):
                psT = psp.tile(
                    [128, 2 * 128], cdt, tag=f"ps_0_{bb % 4}", name=f"psT{bb}"
                )
                for c in range(2):
                    nc.tensor.matmul(
                        psT[:, c * 128 : (c + 1) * 128],
                        h_tiles[(1, c)][:, bb * 128 : (bb + 1) * 128],
                        ident[:, :],
                        is_transpose=True,
                    )
                # evacuate both c halves as one [128, 256] copy
                if bb % 2 == 0:
                    nc.vector.tensor_scalar_add(h2t[:, bb, :], psT[:, :], 0.0)
                else:
                    nc.scalar.copy(h2t[:, bb, :], psT[:, :])

            def step_a(bb):
                psA = psp.tile(
                    [128, 2 * 156], f32, tag=f"ps_1_{bb % 4}", name=f"psA{bb}"
                )
                for jh in range(2):
                    nc.tensor.matmul(
                        psA[:, jh * 156 : (jh + 1) * 156],
                        h2t[:, bb, jh * 128 : (jh + 1) * 128],
                        xd_sb[:, bb * 156 : (bb + 1) * 156],
                        start=True,
                        stop=True,
                    )
                # scatter into i-major G layout (col = i*64 + bb*4 + bi) so
                # step B's stationary slices are contiguous (BIR requires a
                # single free dim on the weights AP)
                for jh in range(2):
                    src = psA[:, jh * 156 : (jh + 1) * 156].rearrange(
                        "p (i w) -> p i w", i=F
                    )
                    dst = g_v[:, jh, :, bb * 4 : (bb + 1) * 4]
                    if bb % 2 == 0:
                        nc.scalar.copy(dst, src)
                    else:
                        nc.vector.tensor_scalar_add(dst, src, 0.0)

            transpose_pair(0)
            transpose_pair(1)
            for bb in range(2, 16):
                transpose_pair(bb)
                step_a(bb - 2)
            step_a(14)
            step_a(15)
            h_reduce(1)   # layer-1 d-sum on GpSimd, in parallel with the tail
            nc.sync.dma_start(out_p[:, 2:4, :], out_sb[:, 2:4, :])

            # step B: out3[b, u] = sum_{i, jh} G'[jh][:, (i, b)]^T
            #                                   @ W2[jh][:, i*256:(i+1)*256]
            psB = psp.tile([BL, U], f32, tag="ps_0_0", name="psB")
            for i in range(F):
                for jh in range(2):
                    nc.tensor.matmul(
                        psB[:, :],
                        g_sb[:, jh, i * BL : (i + 1) * BL],
                        w2sb[:, jh, i * U : (i + 1) * U],
                        start=(i == 0 and jh == 0),
                        stop=(i == F - 1 and jh == 1),
                    )
            nc.vector.tensor_scalar_add(out3_sb[:, :], psB[:, :], 0.0)
            nc.sync.dma_start(out3_p[:, :], out3_sb[:, :])

    nc.compile()
    return nc


def _get_program():
    if "nc" not in _prog_cache:
        _prog_cache["nc"] = _build_program()
    return _prog_cache["nc"]


def _prep_maps(inputs):
    cdt = _np_dt()
    x = np.asarray(inputs["inputs"], np.float32)          # [512, 39, 32]
    Ws = [np.asarray(inputs[f"W{k}"], np.float32) for k in range(3)]
    bs = [np.asarray(inputs[f"b{k}"], np.float32) for k in range(3)]

    # layer-0 symmetric packing: pair slot t*128 + p -> (i, j), i <= j,
    # with the j > i weight row folded in host-side
    pairs = [(i, j) for i in range(F) for j in range(i, F)]
    w0r = Ws[0].reshape(F, F, U)
    w0t = np.zeros((KT0, 128, U), np.float32)
    for s, (i, j) in enumerate(pairs):
        t, p = divmod(s, 128)
        w0t[t, p] = w0r[i, j] if i == j else w0r[i, j] + w0r[j, i]
    w0_tiled = np.ascontiguousarray(w0t.transpose(1, 0, 2).astype(cdt))
    w1_tiled = np.ascontiguousarray(
        Ws[1].reshape(KT12, 128, U).transpose(1, 0, 2).astype(cdt)
    )
    # step-B W2 layout: w2[j, jh, i*256 + u] = W2[(i, jh*128 + j), u]
    w2r = Ws[2].reshape(F, 2, 128, U)                     # [i, jh, j, u]
    w2_tiled = np.ascontiguousarray(
        w2r.transpose(2, 1, 0, 3).reshape(128, 2, F * U).astype(cdt)
    )
    ident = np.ascontiguousarray(np.eye(128, dtype=np.float32).astype(cdt))
    bias = np.zeros((128, 4), np.float32)
    for l in range(2):
        for c in range(2):
            bias[:, l * 2 + c] = bs[l][c * 128 : (c + 1) * 128]

    pr_i = np.array([p[0] for p in pairs])
    pr_j = np.array([p[1] for p in pairs])
    in_maps = []
    for core in range(N_CORES):
        xs = x[core * BL : (core + 1) * BL]               # [64, 39, 32]
        x0T = xs.transpose(1, 0, 2).reshape(F, R).astype(cdt)
        x0r = np.ascontiguousarray(np.repeat(x0T, FP, axis=0))
        xi_all = np.zeros((KT0 * 128, R), cdt)
        xj_all = np.zeros((KT0 * 128, R), cdt)
        xi_all[: len(pairs)] = x0T[pr_i]
        xj_all[: len(pairs)] = x0T[pr_j]
        xij = np.zeros((2 * KT0, 128, R), cdt)
        xij[0::2] = xi_all.reshape(KT0, 128, R)
        xij[1::2] = xj_all.reshape(KT0, 128, R)
        xij = np.ascontiguousarray(xij)
        # step-A x0 block-diagonal: xd[p, bb*156 + i*4 + bi]
        #   = xs[bb*4 + bi, i, p % 32] when p // 32 == bi
        xd = np.zeros((128, 16, F, 4), np.float32)
        xsr = xs.reshape(16, 4, F, D)                     # [bb, bi, i, d]
        for bi in range(4):
            xd[bi * 32 : (bi + 1) * 32, :, :, bi] = xsr[:, bi].transpose(2, 0, 1)
        xd = np.ascontiguousarray(xd.reshape(128, 16 * 156).astype(cdt))
        in_maps.append(
            {
                "xij0": xij,
                "x0r": x0r,
                "w0": w0_tiled,
                "w1": w1_tiled,
                "w2": w2_tiled,
                "xd": xd,
                "ident": ident,
                "bias": bias,
            }
        )
    return in_maps, bs


def _finish_output(results, bs):
    outs = []
    for core in range(N_CORES):
        o = np.asarray(results[core]["out"], np.float32)  # [128, 4, 64]
        o3 = np.asarray(results[core]["out3"], np.float32)  # [64, 256]
        full = np.concatenate(
            [o.transpose(2, 1, 0).reshape(BL, 512), o3], axis=1
        )
        outs.append(full)
    out = np.concatenate(outs, axis=0)
    for l in range(3):
        out[:, l * U : (l + 1) * U] += D * bs[l]
    return np.ascontiguousarray(out.astype(np.float32))


def kernel(**inputs) -> np.ndarray:
    from concourse.bass_utils import run_bass_kernel_spmd

    in_maps, bs = _prep_maps(inputs)
    nc = _get_program()
    res = run_bass_kernel_spmd(nc, in_maps, list(range(N_CORES))).results
    return _finish_output(res, bs)
